# revision 1
# baseline (speedup 1.0000x reference)
"""Causal self-attention (sparse column mask) on 8 Trainium2 NeuronCores.

Problem: B=8, T=1024, C=512, 8 heads (hd=64).
  q/k/v = x @ W{q,k,v}.T + b;  att = softmax(mask(q k^T / 8));  y = att v
  out = y @ Wp.T + bp
Mask: causal lower-triangle, minus every column j with j % 25 == 24.

Sharding: pure data-parallel over batch — core b computes batch element b.

Per-core kernel design (all matmul operands fp16, PSUM accumulation f32):
  - Host pre-transposes x[b] -> xT [C, T] and all weights -> W^T [c_in, c_out],
    so every on-chip matmul has its contraction dim on partitions.
  - Projections produce q^T, k^T [C, T] (heads = partition blocks of 64) and
    v [T, C]. q bias is added during PSUM evacuation (DVE tensor_scalar,
    per-partition); k bias is dropped (softmax shift invariance); v bias is
    folded into the output bias on host (bp' = Wp @ bv + bp, sent broadcast).
  - Attention, phase-separated per query chunk ic (512 wide) so the PE array
    stays in one tiling mode per phase (mode switches drain the PE):
      QK phase (64x128 row-tiled): per head-pair p, per key tile J: two K=64
      matmuls (tile_position (0,0)/(64,0)) -> S^T in 2 PSUM banks; one ACT
      exp call over both (scale=1/8, per-partition bias -30 on j%25==24
      columns) -> fp16 SBUF; causal diagonal zeroed by one fp16 multiply with
      a broadcast lower-triangle tile on DVE.
      AV phase (128x64 col-tiled): per pair, accumulate y'^T and the
      replicated denominators (ones-weight matmuls) over J; then rden via
      approx reciprocal and one tensor_tensor multiply PSUM->SBUF fp16.
  - Output projection consumes y_norm^T directly; bias added during the DVE
    evacuation (tensor_tensor add with a host-broadcast bias tile).
"""

import numpy as np

B, T, C = 8, 1024, 512
H = 8
HD = C // H
P = 128
JD = 25  # joined dim; column j masked when j % 25 == 24
N_CORES = 8
NEG = -30.0  # added post-scale; exp(-30) flushes to 0 in fp16

_CACHE = {}


def _build():
    import concourse.bass as bass
    import concourse.mybir as mybir
    import concourse.tile as tile
    from concourse import bacc

    f16 = mybir.dt.float16
    f32 = mybir.dt.float32
    AF = mybir.ActivationFunctionType
    ALU = mybir.AluOpType

    nc = bacc.Bacc("TRN2", target_bir_lowering=False, debug=False)

    xT = nc.dram_tensor("xT", [C, T], f16, kind="ExternalInput").ap()
    wqT = nc.dram_tensor("wqT", [C, C], f16, kind="ExternalInput").ap()
    wkT = nc.dram_tensor("wkT", [C, C], f16, kind="ExternalInput").ap()
    wvT = nc.dram_tensor("wvT", [C, C], f16, kind="ExternalInput").ap()
    wpT = nc.dram_tensor("wpT", [C, C], f16, kind="ExternalInput").ap()
    bq = nc.dram_tensor("bq", [P, C // P], f32, kind="ExternalInput").ap()
    bppb = nc.dram_tensor("bppb", [P, C], f32, kind="ExternalInput").ap()
    ones64 = nc.dram_tensor("ones64", [P, HD], f16, kind="ExternalInput").ap()
    tri = nc.dram_tensor("tri", [P, P], f16, kind="ExternalInput").ap()
    cmask = nc.dram_tensor("cmask", [P, T // P], f32, kind="ExternalInput").ap()
    out = nc.dram_tensor("out", [T, C], f32, kind="ExternalOutput").ap()

    KT = C // P  # 4 c_in tiles
    MT = C // P  # 4 c_out tiles (= head pairs)
    RT = T // P  # 8 t tiles

    with tile.TileContext(nc) as tc:
        with (
            tc.tile_pool(name="const", bufs=1) as const,
            tc.tile_pool(name="persist", bufs=1) as persist,
            tc.tile_pool(name="es", bufs=24) as es_pool,
            tc.tile_pool(name="rden", bufs=4) as rden_pool,
            tc.tile_pool(name="ot", bufs=4) as ot_pool,
            tc.tile_pool(name="pbig", bufs=2, space="PSUM") as pbig,
            tc.tile_pool(name="psmall", bufs=4, space="PSUM") as psmall,
        ):
            # ---- consolidated input loads (big DMAs, two queues) ----
            def load(shape, dtype, src, tag, eng):
                t = const.tile(shape, dtype, name=tag, tag=tag)
                eng.dma_start(out=t, in_=src)
                return t

            r3 = lambda a: a.rearrange("(a p) n -> p a n", p=P)  # noqa: E731
            # first-needed data in small chunks so the first matmuls start early
            xT_c = [
                load([P, 1, T], f16, r3(xT)[:, k : k + 1, :], f"xT{k}", nc.sync)
                for k in range(KT)
            ]
            wq_c = [
                load([P, 1, C], f16, r3(wqT)[:, k : k + 1, :], f"wqc{k}", nc.scalar)
                for k in range(KT)
            ]
            wk_a = load([P, KT, C], f16, r3(wkT), "wk", nc.scalar)
            tri_s = load([P, P], f16, tri, "tri", nc.sync)
            cmask_s = load([P, T // P], f32, cmask, "cmask", nc.sync)
            wv_a = load([P, KT, C], f16, r3(wvT), "wv", nc.sync)
            bq_s = load([P, C // P], f32, bq, "bq", nc.scalar)
            ones64_s = load([P, HD], f16, ones64, "ones64", nc.scalar)
            wp_a = load([P, KT, C], f16, r3(wpT), "wp", nc.scalar)
            bppb_s = load([P, C], f32, bppb, "bppb", nc.sync)

            def xt(k):
                return xT_c[k][:, 0, :]

            def wq(k):
                return wq_c[k][:, 0, :]

            qT_t = [persist.tile([P, T], f16, name=f"qT{m}", tag=f"qT{m}") for m in range(MT)]
            kT_t = [persist.tile([P, T], f16, name=f"kT{m}", tag=f"kT{m}") for m in range(MT)]
            v_t = [persist.tile([P, C], f16, name=f"v{r}", tag=f"v{r}") for r in range(RT)]
            yn_t = [persist.tile([P, T], f16, name=f"yn{m}", tag=f"yn{m}") for m in range(MT)]

            # broadcast lower-triangle tile across both heads of an es tile
            tri_b = bass.AP(
                tensor=tri_s.tensor,
                offset=tri_s.offset,
                ap=[list(tri_s.ap[0]), [0, 2], list(tri_s.ap[1])],
            )

            # ---- emission helpers ----
            def proj_qk(m):
                for which, dst, biased in (("q", qT_t[m], True), ("k", kT_t[m], False)):
                    ps = pbig.tile([P, T], f32, name="psqk", tag="pbig")
                    for half in range(2):
                        o = ps[:, 512 * half : 512 * (half + 1)]
                        for k in range(KT):
                            w_ap = (
                                wq(k)[:, P * m : P * (m + 1)]
                                if which == "q"
                                else wk_a[:, k, P * m : P * (m + 1)]
                            )
                            nc.tensor.matmul(
                                o,
                                lhsT=w_ap,
                                rhs=xt(k)[:, 512 * half : 512 * (half + 1)],
                                start=(k == 0),
                                stop=(k == KT - 1),
                            )
                    if biased:
                        nc.vector.tensor_scalar_add(dst, ps, bq_s[:, m : m + 1])
                    else:
                        nc.vector.tensor_copy(dst, ps)

            def proj_v(r0, r1):
                for r in range(r0, r1):
                    ps = psmall.tile([P, C], f32, name="pv", tag="sm")
                    for k in range(KT):
                        nc.tensor.matmul(
                            ps,
                            lhsT=xt(k)[:, P * r : P * (r + 1)],
                            rhs=wv_a[:, k, :],
                            start=(k == 0),
                            stop=(k == KT - 1),
                        )
                    nc.scalar.activation(v_t[r], ps, AF.Copy)

            es_t = {}

            def qk_phase(ic, p):
                for J in range(4 * (ic + 1)):
                    i0 = max(512 * ic, P * J)
                    w = 512 * (ic + 1) - i0
                    st = pbig.tile([P, 2, 512], f32, name="st", tag="pbig")
                    for h in range(2):
                        nc.tensor.matmul(
                            st[:, h, :w],
                            lhsT=kT_t[p][64 * h : 64 * (h + 1), P * J : P * (J + 1)],
                            rhs=qT_t[p][64 * h : 64 * (h + 1), i0 : i0 + w],
                            start=True,
                            stop=True,
                            tile_position=(64 * h, 0),
                        )
                    es = es_pool.tile([P, 2, 512], f16, name="es", tag="es")
                    es_t[(ic, p, J)] = es
                    nc.scalar.activation(
                        es[:, :, :w],
                        st[:, :, :w],
                        AF.Exp,
                        bias=cmask_s[:, J : J + 1],
                        scale=0.125,
                    )
                    if P * J >= 512 * ic:  # diagonal: zero the causal triangle
                        nc.vector.tensor_tensor(
                            out=es[:, :, :P], in0=es[:, :, :P], in1=tri_b, op=ALU.mult
                        )

            def av_phase(ic, p):
                av = psmall.tile([P, 512], f32, name="av", tag="sm")
                den = psmall.tile([P, 512], f32, name="den", tag="sm")
                nJ = 4 * (ic + 1)
                for J in range(nJ):
                    i0 = max(512 * ic, P * J)
                    w = 512 * (ic + 1) - i0
                    io = i0 - 512 * ic
                    first, last = J == 0, J == nJ - 1
                    es = es_t.pop((ic, p, J))
                    for h in range(2):
                        nc.tensor.matmul(
                            av[64 * h : 64 * (h + 1), io : io + w],
                            lhsT=v_t[J][:, P * p + 64 * h : P * p + 64 * (h + 1)],
                            rhs=es[:, h, :w],
                            start=first,
                            stop=last,
                            tile_position=(0, 64 * h),
                        )
                        nc.tensor.matmul(
                            den[64 * h : 64 * (h + 1), io : io + w],
                            lhsT=ones64_s,
                            rhs=es[:, h, :w],
                            start=first,
                            stop=last,
                            tile_position=(0, 64 * h),
                        )
                rden = rden_pool.tile([P, 512], f32, name="rden", tag="rden")
                nc.vector.reciprocal_approx_fast(out=rden, in_=den)
                nc.vector.tensor_mul(yn_t[p][:, 512 * ic : 512 * (ic + 1)], av, rden)

            po_t = {}

            def outproj_start(r0, r1, mhi):
                for r in range(r0, r1):
                    po = po_t.setdefault(
                        r, psmall.tile([P, C], f32, name=f"po{r}", tag="sm")
                    )
                    for m in range(mhi):
                        nc.tensor.matmul(
                            po,
                            lhsT=yn_t[m][:, P * r : P * (r + 1)],
                            rhs=wp_a[:, m, :],
                            start=(m == 0),
                            stop=False,
                        )

            def outproj_finish(r0, r1, mlo):
                for r in range(r0, r1):
                    po = po_t.pop(r)
                    for m in range(mlo, MT):
                        nc.tensor.matmul(
                            po,
                            lhsT=yn_t[m][:, P * r : P * (r + 1)],
                            rhs=wp_a[:, m, :],
                            start=False,
                            stop=(m == MT - 1),
                        )
                    ot = ot_pool.tile([P, C], f32, name="ot", tag="ot")
                    nc.vector.tensor_tensor(out=ot, in0=po, in1=bppb_s, op=ALU.add)
                    nc.sync.dma_start(out=out[P * r : P * (r + 1), :], in_=ot)

            def outproj(r0, r1):
                outproj_start(r0, r1, 3)
                outproj_finish(r0, r1, 3)

            # ---- emission schedule: weave attention into projections so the
            # ACT exp pipeline starts early and never starves ----
            proj_qk(0)
            qk_phase(0, 0)
            proj_qk(1)
            proj_v(0, 4)
            qk_phase(0, 1)
            av_phase(0, 0)
            proj_qk(2)
            proj_v(4, 8)
            qk_phase(0, 2)
            av_phase(0, 1)
            proj_qk(3)
            qk_phase(0, 3)
            av_phase(0, 2)
            av_phase(0, 3)
            qk_phase(1, 0)
            outproj(0, 4)
            qk_phase(1, 1)
            av_phase(1, 0)
            qk_phase(1, 2)
            av_phase(1, 1)
            qk_phase(1, 3)
            av_phase(1, 2)
            av_phase(1, 3)
            outproj(4, 8)

    nc.compile()
    return nc


def _prep_inputs(x, Wq, bq, Wk, bk, Wv, bv, Wp, bp):
    """Host-side prep: transposes, bias folding, mask tables. Returns in_maps."""
    f16 = np.float16
    wqT = np.ascontiguousarray(Wq.T).astype(f16)
    wkT = np.ascontiguousarray(Wk.T).astype(f16)
    wvT = np.ascontiguousarray(Wv.T).astype(f16)
    wpT = np.ascontiguousarray(Wp.T).astype(f16)
    bq_pp = np.ascontiguousarray(bq.astype(np.float32).reshape(C // P, P).T)
    # v bias folds into output bias: out = (y' + bv) @ Wp.T + bp
    bpp = (
        Wp.astype(np.float64) @ bv.astype(np.float64) + bp.astype(np.float64)
    ).astype(np.float32)
    bppb = np.broadcast_to(bpp[None, :], (P, C)).copy()
    ones64 = np.ones((P, HD), dtype=f16)
    tri = (np.arange(P)[:, None] <= np.arange(P)[None, :]).astype(f16)  # keep j<=i
    j_idx = np.arange(P)[:, None] + P * np.arange(T // P)[None, :]
    cmask = np.where(j_idx % JD == JD - 1, np.float32(NEG), np.float32(0.0)).astype(
        np.float32
    )

    shared = {
        "wqT": wqT,
        "wkT": wkT,
        "wvT": wvT,
        "wpT": wpT,
        "bq": bq_pp,
        "bppb": bppb,
        "ones64": ones64,
        "tri": tri,
        "cmask": cmask,
    }
    in_maps = []
    for b in range(N_CORES):
        m = dict(shared)
        m["xT"] = np.ascontiguousarray(x[b].T).astype(f16)
        in_maps.append(m)
    return in_maps


def kernel(x, Wq, bq, Wk, bk, Wv, bv, Wp, bp):
    from concourse import bass_utils

    x = np.asarray(x, dtype=np.float32)
    if "nc" not in _CACHE:
        _CACHE["nc"] = _build()
    nc = _CACHE["nc"]
    in_maps = _prep_inputs(
        x,
        np.asarray(Wq, np.float32),
        np.asarray(bq, np.float32),
        np.asarray(Wk, np.float32),
        np.asarray(bk, np.float32),
        np.asarray(Wv, np.float32),
        np.asarray(bv, np.float32),
        np.asarray(Wp, np.float32),
        np.asarray(bp, np.float32),
    )
    res = bass_utils.run_bass_kernel_spmd(nc, in_maps, core_ids=list(range(N_CORES)))
    return np.stack([res.results[b]["out"] for b in range(N_CORES)], axis=0)



# revision 5
# speedup vs baseline: 1.0369x; 1.0369x over previous
"""Causal self-attention (sparse column mask) on 8 Trainium2 NeuronCores.

Problem: B=8, T=1024, C=512, 8 heads (hd=64).
  q/k/v = x @ W{q,k,v}.T + b;  att = softmax(mask(q k^T / 8));  y = att v
  out = y @ Wp.T + bp
Mask: causal lower-triangle, minus every column j with j % 25 == 24.

Sharding: pure data-parallel over batch - core b computes batch element b.

v2 design (fp16 matmul operands, f32 PSUM). Same math as v1; restructured
schedule so the PE stream is dense from ~2us on:
  - k-outer projection start: the first q/k projections consume x/W DMA
    chunks as they arrive instead of waiting for the full tensors.
  - QK tile pairs run concurrently in the PE array (row groups 0/64); AV and
    the ones-matmul denominators run as concurrent column-group pairs.
  - Fine-grained interleave: qk/av/proj/outproj matmuls are woven so the PE
    never has to wait for ACT exp between QK tiles (st PSUM recycling).
  - Output projection r0..3 only needs the first query half -> runs as filler
    during the second attention chunk; r4..7 are pre-accumulated (m=0..2)
    into PSUM banks freed by the last QK phase, so the tail after the final
    AV is just 4 matmuls + evac + small fp16 DMAs.
  - Output written fp16 (host upcasts); per-row-tile DMA on the gpsimd ring.
"""

import numpy as np

B, T, C = 8, 1024, 512
H = 8
HD = C // H
P = 128
JD = 25  # joined dim; column j masked when j % 25 == 24
N_CORES = 8
NEG = -30.0  # added post-scale; exp(-30) flushes to 0 in fp16

_CACHE = {}


def _build():
    import concourse.bass as bass
    import concourse.mybir as mybir
    import concourse.tile as tile
    from concourse import bacc

    f16 = mybir.dt.float16
    f32 = mybir.dt.float32
    AF = mybir.ActivationFunctionType
    ALU = mybir.AluOpType

    nc = bacc.Bacc("TRN2", target_bir_lowering=False, debug=False)

    xT = nc.dram_tensor("xT", [C, T], f16, kind="ExternalInput").ap()
    wqT = nc.dram_tensor("wqT", [C, C], f16, kind="ExternalInput").ap()
    wkT = nc.dram_tensor("wkT", [C, C], f16, kind="ExternalInput").ap()
    wvT = nc.dram_tensor("wvT", [C, C], f16, kind="ExternalInput").ap()
    wpT = nc.dram_tensor("wpT", [C, C], f16, kind="ExternalInput").ap()
    bq = nc.dram_tensor("bq", [P, C // P], f32, kind="ExternalInput").ap()
    bppb = nc.dram_tensor("bppb", [P, C], f32, kind="ExternalInput").ap()
    ones64 = nc.dram_tensor("ones64", [P, HD], f16, kind="ExternalInput").ap()
    tri = nc.dram_tensor("tri", [P, P], f16, kind="ExternalInput").ap()
    cmask = nc.dram_tensor("cmask", [P, T // P], f32, kind="ExternalInput").ap()
    out = nc.dram_tensor("out", [T, C], f16, kind="ExternalOutput").ap()

    KT = C // P  # 4 c_in tiles
    MT = C // P  # 4 c_out tiles (= head pairs)
    RT = T // P  # 8 t tiles

    with tile.TileContext(nc) as tc:
        with (
            tc.tile_pool(name="const", bufs=1) as const,
            tc.tile_pool(name="persist", bufs=1) as persist,
            tc.tile_pool(name="es", bufs=16) as es_pool,
            tc.tile_pool(name="rden", bufs=4) as rden_pool,
            tc.tile_pool(name="ot", bufs=4) as ot_pool,
            tc.tile_pool(name="stp", bufs=2, space="PSUM") as stp,
            tc.tile_pool(name="work", bufs=4, space="PSUM") as work,
        ):
            # ---- input DMAs: chunked, ordered by first consumption ----
            def load(shape, dtype, src, tag, eng):
                t = const.tile(shape, dtype, name=tag, tag=tag)
                eng.dma_start(out=t, in_=src)
                return t

            r3 = lambda a: a.rearrange("(a p) n -> p a n", p=P)  # noqa: E731
            # sync (HWDGE) ring, ordered by first consumption: per-k triples
            # (xT half0 chunk, wq chunk, wk chunk), then wv, then xT half1.
            xT_c = [[None] * 2 for _ in range(KT)]
            wq_c, wk_c = [None] * KT, [None] * KT
            for k in range(KT):
                xT_c[k][0] = load(
                    [P, 1, 512], f16, r3(xT)[:, k : k + 1, 0:512], f"xT{k}h0", nc.sync
                )
                wq_c[k] = load([P, 1, C], f16, r3(wqT)[:, k : k + 1, :], f"wq{k}", nc.sync)
                wk_c[k] = load([P, 1, C], f16, r3(wkT)[:, k : k + 1, :], f"wk{k}", nc.sync)
            wv_a = load([P, KT, C], f16, r3(wvT), "wv", nc.sync)
            for k in range(KT):
                xT_c[k][1] = load(
                    [P, 1, 512], f16, r3(xT)[:, k : k + 1, 512:1024], f"xT{k}h1", nc.sync
                )
            # small consts on the scalar (ACT) ring: done before the first exp
            tri_s = load([P, P], f16, tri, "tri", nc.scalar)
            cmask_s = load([P, T // P], f32, cmask, "cmask", nc.scalar)
            bq_s = load([P, C // P], f32, bq, "bq", nc.scalar)
            ones64_s = load([P, HD], f16, ones64, "ones64", nc.scalar)
            # late-needed bulk on the gpsimd software-DGE ring
            bppb_s = load([P, C], f32, bppb, "bppb", nc.gpsimd)
            wp_a = load([P, KT, C], f16, r3(wpT), "wp", nc.gpsimd)

            def xt(k, h):
                return xT_c[k][h][:, 0, :]

            qT_t = [persist.tile([P, T], f16, name=f"qT{m}", tag=f"qT{m}") for m in range(MT)]
            kT_t = [persist.tile([P, T], f16, name=f"kT{m}", tag=f"kT{m}") for m in range(MT)]
            v_t = [persist.tile([P, C], f16, name=f"v{r}", tag=f"v{r}") for r in range(RT)]
            yn_t = [persist.tile([P, T], f16, name=f"yn{m}", tag=f"yn{m}") for m in range(MT)]

            # broadcast lower-triangle tile across both heads of an es tile
            tri_b = bass.AP(
                tensor=tri_s.tensor,
                offset=tri_s.offset,
                ap=[list(tri_s.ap[0]), [0, 2], list(tri_s.ap[1])],
            )

            # ---- emission helpers ----
            def proj_qk_m(m, h):
                """q and k projections for pair m, query half h (k-outer)."""
                qs = work.tile([P, 512], f32, name="qps", tag="wk")
                ks = work.tile([P, 512], f32, name="kps", tag="wk")
                for k in range(KT):
                    nc.tensor.matmul(
                        qs,
                        lhsT=wq_c[k][:, 0, P * m : P * (m + 1)],
                        rhs=xt(k, h),
                        start=(k == 0),
                        stop=(k == KT - 1),
                    )
                    nc.tensor.matmul(
                        ks,
                        lhsT=wk_c[k][:, 0, P * m : P * (m + 1)],
                        rhs=xt(k, h),
                        start=(k == 0),
                        stop=(k == KT - 1),
                    )
                sl = slice(512 * h, 512 * (h + 1))
                nc.vector.tensor_scalar_add(qT_t[m][:, sl], qs, bq_s[:, m : m + 1])
                nc.vector.tensor_copy(kT_t[m][:, sl], ks)

            def proj_v(r):
                pv = work.tile([P, C], f32, name="pv", tag="wk")
                for k in range(KT):
                    nc.tensor.matmul(
                        pv,
                        lhsT=xT_c[k][r // 4][:, 0, P * (r % 4) : P * (r % 4 + 1)],
                        rhs=wv_a[:, k, :],
                        start=(k == 0),
                        stop=(k == KT - 1),
                    )
                nc.vector.tensor_copy(v_t[r], pv)

            es_t = {}

            def qk2(ic, p, jp):
                """two QK key-tiles (J=2jp, 2jp+1) + exp (+ causal tri)."""
                for J in (2 * jp, 2 * jp + 1):
                    if J >= 4 * (ic + 1):
                        continue
                    i0 = max(512 * ic, P * J)
                    w = 512 * (ic + 1) - i0
                    st = stp.tile([P, 2, 512], f32, name="st", tag="st")
                    for h in range(2):
                        nc.tensor.matmul(
                            st[:, h, :w],
                            lhsT=kT_t[p][64 * h : 64 * (h + 1), P * J : P * (J + 1)],
                            rhs=qT_t[p][64 * h : 64 * (h + 1), i0 : i0 + w],
                            start=True,
                            stop=True,
                            tile_position=(64 * h, 0),
                        )
                    es = es_pool.tile([P, 2, 512], f16, name="es", tag="es")
                    es_t[(ic, p, J)] = es
                    nc.scalar.activation(
                        es[:, :, :w],
                        st[:, :, :w],
                        AF.Exp,
                        bias=cmask_s[:, J : J + 1],
                        scale=0.125,
                    )
                    if P * J >= 512 * ic:  # diagonal: zero the causal triangle
                        nc.vector.tensor_tensor(
                            out=es[:, :, :P], in0=es[:, :, :P], in1=tri_b, op=ALU.mult
                        )

            av_ps = {}

            def av2(ic, p, jp):
                """two AV+den key-tiles for (ic, p)."""
                nJ = 4 * (ic + 1)
                if jp == 0:
                    av_ps[(ic, p)] = (
                        work.tile([P, 512], f32, name="av", tag="wk"),
                        work.tile([P, 512], f32, name="den", tag="wk"),
                    )
                av, den = av_ps[(ic, p)]
                for J in (2 * jp, 2 * jp + 1):
                    if J >= nJ:
                        continue
                    i0 = max(512 * ic, P * J)
                    w = 512 * (ic + 1) - i0
                    io = i0 - 512 * ic
                    first, last = J == 0, J == nJ - 1
                    es = es_t.pop((ic, p, J))
                    for h in range(2):
                        nc.tensor.matmul(
                            av[64 * h : 64 * (h + 1), io : io + w],
                            lhsT=v_t[J][:, P * p + 64 * h : P * p + 64 * (h + 1)],
                            rhs=es[:, h, :w],
                            start=first,
                            stop=last,
                            tile_position=(0, 64 * h),
                        )
                        nc.tensor.matmul(
                            den[64 * h : 64 * (h + 1), io : io + w],
                            lhsT=ones64_s,
                            rhs=es[:, h, :w],
                            start=first,
                            stop=last,
                            tile_position=(0, 64 * h),
                        )

            def rden_mul(ic, p):
                av, den = av_ps.pop((ic, p))
                rden = rden_pool.tile([P, 512], f32, name="rden", tag="rden")
                nc.vector.reciprocal_approx_fast(out=rden, in_=den)
                nc.vector.tensor_mul(yn_t[p][:, 512 * ic : 512 * (ic + 1)], av, rden)

            def av4(ic, p):
                av2(ic, p, 0)
                av2(ic, p, 1)
                rden_mul(ic, p)

            def op_emit(r, po, m0, m1, evac):
                """outproj matmuls m0..m1 for row-tile r into po; evac+DMA if done."""
                for m in range(m0, m1 + 1):
                    nc.tensor.matmul(
                        po,
                        lhsT=yn_t[m][:, P * r : P * (r + 1)],
                        rhs=wp_a[:, m, :],
                        start=(m == 0),
                        stop=(m == MT - 1),
                    )
                if evac:
                    ot = ot_pool.tile([P, C], f16, name="ot", tag="ot")
                    nc.vector.tensor_tensor(out=ot, in0=po, in1=bppb_s, op=ALU.add)
                    nc.sync.dma_start(out=out[P * r : P * (r + 1), :], in_=ot)

            def op_full(r):
                po = work.tile([P, C], f32, name=f"po{r}", tag="wk")
                op_emit(r, po, 0, MT - 1, True)

            # ================= emission schedule =================
            # A: projections h0 woven with the first QK tiles
            proj_qk_m(0, 0)
            qk2(0, 0, 0)
            proj_qk_m(1, 0)
            qk2(0, 0, 1)
            proj_qk_m(2, 0)
            qk2(0, 1, 0)
            proj_qk_m(3, 0)
            qk2(0, 1, 1)
            # B: ic=0 attention + v + h1 projections
            proj_v(0)
            proj_v(1)
            qk2(0, 2, 0)
            proj_v(2)
            proj_v(3)
            qk2(0, 2, 1)
            av4(0, 0)
            qk2(0, 3, 0)
            proj_qk_m(0, 1)
            qk2(0, 3, 1)
            av4(0, 1)
            proj_qk_m(1, 1)
            av4(0, 2)
            proj_qk_m(2, 1)
            av4(0, 3)
            proj_qk_m(3, 1)
            # C: ic=1 attention; av lags qk by one phase; outproj r0..3 and the
            # remaining v tiles fill the gaps.
            qk2(1, 0, 0)
            op_full(0)
            qk2(1, 0, 1)
            op_full(1)
            qk2(1, 0, 2)
            proj_v(4)
            proj_v(5)
            qk2(1, 0, 3)
            proj_v(6)
            proj_v(7)
            for p in (1, 2, 3):
                for jp in range(4):
                    qk2(1, p, jp)
                    av2(1, p - 1, jp)
                    if p == 1 and jp == 0:
                        op_full(2)
                    if p == 1 and jp == 1:
                        op_full(3)
                rden_mul(1, p - 1)
            # tail: av(1,3) woven with pre-accumulated outproj r4..7 (m=0..2)
            # in the PSUM banks freed by the last QK phase.
            po45 = stp.tile([P, 2, 512], f32, name="po45", tag="st")
            po67 = stp.tile([P, 2, 512], f32, name="po67", tag="st")
            po_hi = {4: po45[:, 0, :], 5: po45[:, 1, :], 6: po67[:, 0, :], 7: po67[:, 1, :]}
            av2(1, 3, 0)
            op_emit(4, po_hi[4], 0, 2, False)
            op_emit(5, po_hi[5], 0, 2, False)
            av2(1, 3, 1)
            op_emit(6, po_hi[6], 0, 2, False)
            op_emit(7, po_hi[7], 0, 2, False)
            av2(1, 3, 2)
            av2(1, 3, 3)
            rden_mul(1, 3)
            for r in range(4, 8):
                op_emit(r, po_hi[r], 3, 3, True)

    nc.compile()
    return nc


def _prep_inputs(x, Wq, bq, Wk, bk, Wv, bv, Wp, bp):
    """Host-side prep: transposes, bias folding, mask tables. Returns in_maps."""
    f16 = np.float16
    wqT = np.ascontiguousarray(Wq.T).astype(f16)
    wkT = np.ascontiguousarray(Wk.T).astype(f16)
    wvT = np.ascontiguousarray(Wv.T).astype(f16)
    wpT = np.ascontiguousarray(Wp.T).astype(f16)
    bq_pp = np.ascontiguousarray(bq.astype(np.float32).reshape(C // P, P).T)
    # v bias folds into output bias: out = (y' + bv) @ Wp.T + bp
    bpp = (
        Wp.astype(np.float64) @ bv.astype(np.float64) + bp.astype(np.float64)
    ).astype(np.float32)
    bppb = np.broadcast_to(bpp[None, :], (P, C)).copy()
    ones64 = np.ones((P, HD), dtype=f16)
    tri = (np.arange(P)[:, None] <= np.arange(P)[None, :]).astype(f16)  # keep j<=i
    j_idx = np.arange(P)[:, None] + P * np.arange(T // P)[None, :]
    cmask = np.where(j_idx % JD == JD - 1, np.float32(NEG), np.float32(0.0)).astype(
        np.float32
    )

    shared = {
        "wqT": wqT,
        "wkT": wkT,
        "wvT": wvT,
        "wpT": wpT,
        "bq": bq_pp,
        "bppb": bppb,
        "ones64": ones64,
        "tri": tri,
        "cmask": cmask,
    }
    in_maps = []
    for b in range(N_CORES):
        m = dict(shared)
        m["xT"] = np.ascontiguousarray(x[b].T).astype(f16)
        in_maps.append(m)
    return in_maps


def kernel(x, Wq, bq, Wk, bk, Wv, bv, Wp, bp):
    from concourse import bass_utils

    x = np.asarray(x, dtype=np.float32)
    if "nc" not in _CACHE:
        _CACHE["nc"] = _build()
    nc = _CACHE["nc"]
    in_maps = _prep_inputs(
        x,
        np.asarray(Wq, np.float32),
        np.asarray(bq, np.float32),
        np.asarray(Wk, np.float32),
        np.asarray(bk, np.float32),
        np.asarray(Wv, np.float32),
        np.asarray(bv, np.float32),
        np.asarray(Wp, np.float32),
        np.asarray(bp, np.float32),
    )
    res = bass_utils.run_bass_kernel_spmd(nc, in_maps, core_ids=list(range(N_CORES)))
    return np.stack(
        [res.results[b]["out"].astype(np.float32) for b in range(N_CORES)], axis=0
    )
